# revision 3
# baseline (speedup 1.0000x reference)
"""Trainium2 Bass kernel for the EMAMixer problem.

Semantics (stable form of the reference):
    for each m:  u_t = a_m * u_{t-1} + s_m * x_t,   u_{-1} = -bias_m / s_m
    y[t, e, m] = u_t[e] + bias_m / s_m
    out = y.reshape(B, T, E*M) @ W + b

Implementation per core (data-parallel over B, core c handles batch c):
  - T split into 16 chunks of 128. Per chunk and e-tile (128 features):
    one f32r matmul  x_chunk[:, et].T @ L_cat  ->  psum[e, (m,i)]
    where L_cat[j, m*128+i] = s_m * a_m^(i-j) for j<=i (in-chunk scan as matmul;
    output lands feature-major, i.e. already transposed for the projection).
  - carry across chunks: y = decay*(carry) + psum  via one DVE
    scalar_tensor_tensor per (e-tile, m); carry = col 127 of prev chunk's y.
  - projection: out_psum[i, n] += y_tile.T @ W_tile accumulated over all 32
    (e-tile, m) pairs, two 512-wide PSUM halves; then + cvec (bias + carry-shift
    correction) on DVE, DMA out.
All matmuls run in float32r (full-rate fp32 path on trn2).
"""

import numpy as np

B, T, E, M = 8, 2048, 1024, 4
P = 128                      # partition / chunk size
CH = T // P                  # 16 chunks
ET = E // P                  # 8 e-tiles
NC = 8                       # cores
NO = E                       # output features

_cache: dict = {}


def _sigmoid(x):
    return 1.0 / (1.0 + np.exp(-x))


def _host_consts(alpha_logits, bias, W, b):
    """Precompute small constant tensors in fp64, cast to fp32."""
    a = _sigmoid(alpha_logits.astype(np.float64))            # (M,)
    s = 1.0 - a
    denom = np.maximum(s, 1e-8)
    binf = bias.astype(np.float64) / denom                   # (M,)

    i_idx = np.arange(P)
    # L_cat[j, m*128+i] = s_m * a_m^(i-j)  for j <= i, else 0
    diff = i_idx[None, :] - i_idx[:, None]                   # (j, i) = i - j
    lcat = np.zeros((P, M * P), dtype=np.float64)
    for m in range(M):
        Lm = np.where(diff >= 0, s[m] * np.power(a[m], np.maximum(diff, 0)), 0.0)
        lcat[:, m * P:(m + 1) * P] = Lm
    # dcat[p, m*128+i] = a_m^(i+1)   (broadcast over partitions)
    dcat = np.zeros((P, M * P), dtype=np.float64)
    for m in range(M):
        dcat[:, m * P:(m + 1) * P] = np.power(a[m], i_idx + 1)[None, :]
    # cinit[p, m] = -binf_m
    cinit = np.broadcast_to(-binf[None, :], (P, M)).copy()
    # cvec[n] = sum_m binf_m * sum_e W[e*M+m, n]  + b[n]
    Wf = W.astype(np.float64)
    cvec = b.astype(np.float64).copy()
    for m in range(M):
        cvec += binf[m] * Wf[m::M, :].sum(axis=0)
    cvecb = np.broadcast_to(cvec[None, :], (P, NO)).copy()
    # wt[g=et*M+m, e, :] = W[(et*128+e)*M + m, :]
    wt = W.reshape(ET, P, M, NO).transpose(0, 2, 1, 3).reshape(ET * M, P, NO)

    return (
        lcat.astype(np.float32),
        dcat.astype(np.float32),
        cinit.astype(np.float32),
        cvecb.astype(np.float32),
        np.ascontiguousarray(wt).astype(np.float32),
    )


def _build(mm_dtype="f32r"):
    import concourse.bacc as bacc
    import concourse.tile as tile
    import concourse.mybir as mybir

    key = ("nc", mm_dtype)
    if key in _cache:
        return _cache[key]

    dt_mm = mybir.dt.float32r if mm_dtype == "f32r" else mybir.dt.float32
    f32 = mybir.dt.float32
    mult = mybir.AluOpType.mult
    add = mybir.AluOpType.add

    nc = bacc.Bacc("TRN2", target_bir_lowering=False, debug=False,
                   enable_asserts=False, num_devices=NC)

    xb = nc.dram_tensor("xb", [T, E], dt_mm, kind="ExternalInput").ap()
    wtd = nc.dram_tensor("wtd", [ET * M, P, NO], dt_mm, kind="ExternalInput").ap()
    lcat_d = nc.dram_tensor("lcat", [P, M * P], dt_mm, kind="ExternalInput").ap()
    dcat_d = nc.dram_tensor("dcat", [P, M * P], f32, kind="ExternalInput").ap()
    cinit_d = nc.dram_tensor("cinit", [P, M], f32, kind="ExternalInput").ap()
    cvecb_d = nc.dram_tensor("cvecb", [P, NO], f32, kind="ExternalInput").ap()
    out_d = nc.dram_tensor("out", [T, NO], f32, kind="ExternalOutput").ap()

    with tile.TileContext(nc) as tc:
        with (
            tc.tile_pool(name="consts", bufs=1) as cpool,
            tc.tile_pool(name="wpool", bufs=1) as wpool,
            tc.tile_pool(name="xpool", bufs=2) as xpool,
            tc.tile_pool(name="ypool", bufs=2) as ypool,
            tc.tile_pool(name="opool", bufs=2) as opool,
            tc.tile_pool(name="epsum", bufs=3, space="PSUM") as epsum,
            tc.tile_pool(name="ppsum", bufs=2, space="PSUM") as ppsum,
        ):
            lcat_t = cpool.tile([P, M * P], dt_mm)
            nc.sync.dma_start(out=lcat_t[:], in_=lcat_d[:])
            dcat_t = cpool.tile([P, M * P], f32)
            nc.sync.dma_start(out=dcat_t[:], in_=dcat_d[:])
            cinit_t = cpool.tile([P, M], f32)
            nc.sync.dma_start(out=cinit_t[:], in_=cinit_d[:])
            cvecb_t = cpool.tile([P, NO], f32)
            nc.sync.dma_start(out=cvecb_t[:], in_=cvecb_d[:])

            wt_t = []
            for g in range(ET * M):
                w = wpool.tile([P, NO], dt_mm, tag=f"w{g}")
                nc.sync.dma_start(out=w[:], in_=wtd[g])
                wt_t.append(w)

            y_prev = None
            for c in range(CH):
                xt = xpool.tile([P, E], dt_mm)
                nc.sync.dma_start(out=xt[:], in_=xb[c * P:(c + 1) * P, :])

                y_cur = []
                for et in range(ET):
                    pe = epsum.tile([P, M * P], f32)
                    nc.tensor.matmul(
                        pe[:], xt[:, et * P:(et + 1) * P], lcat_t[:],
                        start=True, stop=True,
                    )
                    yt = ypool.tile([P, M * P], dt_mm, tag=f"y{et}")
                    for m in range(M):
                        sl = slice(m * P, (m + 1) * P)
                        if c == 0:
                            carry = cinit_t[:, m:m + 1]
                        else:
                            carry = y_prev[et][:, m * P + P - 1:m * P + P]
                        nc.vector.scalar_tensor_tensor(
                            out=yt[:, sl], in0=dcat_t[:, sl], scalar=carry,
                            in1=pe[:, sl], op0=mult, op1=add,
                        )
                    y_cur.append(yt)

                ot = opool.tile([P, NO], f32)
                for h in range(2):
                    po = ppsum.tile([P, NO // 2], f32, tag=f"po{h}")
                    k = 0
                    for et in range(ET):
                        for m in range(M):
                            nc.tensor.matmul(
                                po[:],
                                y_cur[et][:, m * P:(m + 1) * P],
                                wt_t[et * M + m][:, h * (NO // 2):(h + 1) * (NO // 2)],
                                start=(k == 0), stop=(k == ET * M - 1),
                            )
                            k += 1
                    nc.vector.tensor_add(
                        ot[:, h * (NO // 2):(h + 1) * (NO // 2)], po[:],
                        cvecb_t[:, h * (NO // 2):(h + 1) * (NO // 2)],
                    )
                nc.sync.dma_start(out=out_d[c * P:(c + 1) * P, :], in_=ot[:])
                y_prev = y_cur

    nc.compile()
    _cache[key] = nc
    return nc


def _in_maps(inputs):
    x = np.ascontiguousarray(inputs["x"], dtype=np.float32)
    assert x.shape == (B, T, E), x.shape
    lcat, dcat, cinit, cvecb, wt = _host_consts(
        np.asarray(inputs["alpha_logits"], np.float32),
        np.asarray(inputs["bias"], np.float32),
        np.ascontiguousarray(inputs["W"], np.float32),
        np.asarray(inputs["b"], np.float32))
    in_maps = []
    for c in range(NC):
        in_maps.append({
            "xb": x[c],
            "wtd": wt,
            "lcat": lcat,
            "dcat": dcat,
            "cinit": cinit,
            "cvecb": cvecb,
        })
    return in_maps


def kernel(x, alpha_logits, bias, W, b):
    from concourse.bass_utils import run_bass_kernel_spmd

    in_maps = _in_maps({"x": x, "alpha_logits": alpha_logits, "bias": bias,
                        "W": W, "b": b})
    nc = _build()
    r = run_bass_kernel_spmd(nc, in_maps, list(range(NC)))
    out = np.stack([r.results[c]["out"] for c in range(NC)], axis=0)
    return out


# revision 7
# speedup vs baseline: 10.3236x; 10.3236x over previous
"""Trainium2 Bass kernel for the EMAMixer problem.

Semantics (stable form of the reference):
    for each m:  u_t = a_m * u_{t-1} + s_m * x_t,   u_{-1} = -bias_m / s_m
    y[t, e, m] = u_t[e] + bias_m / s_m
    out = y.reshape(B, T, E*M) @ W + b

Implementation per core (data-parallel over B, core c handles batch c):
  - T split into 16 chunks of 128. Per chunk and e-tile (128 features):
    one f32r matmul  x_chunk[:, et].T @ L_cat  ->  psum[e, (m,i)]
    where L_cat[j, m*128+i] = s_m * a_m^(i-j) for j<=i (in-chunk scan as matmul;
    output lands feature-major, i.e. already transposed for the projection).
  - carry across chunks: y = decay*(carry) + psum  via one DVE
    scalar_tensor_tensor per (e-tile, m); carry = col 127 of prev chunk's y.
  - projection: out_psum[i, n] += y_tile.T @ W_tile accumulated over all 32
    (e-tile, m) pairs, two 512-wide PSUM halves; then + cvec (bias + carry-shift
    correction) on DVE, DMA out.
All matmuls run in float32r (full-rate fp32 path on trn2).
"""

import numpy as np

B, T, E, M = 8, 2048, 1024, 4
P = 128                      # partition / chunk size
CH = T // P                  # 16 chunks
ET = E // P                  # 8 e-tiles
NC = 8                       # cores
NO = E                       # output features

_cache: dict = {}


def _sigmoid(x):
    return 1.0 / (1.0 + np.exp(-x))


def _host_consts(alpha_logits, bias, W, b):
    """Precompute small constant tensors in fp64, cast to fp32."""
    a = _sigmoid(alpha_logits.astype(np.float64))            # (M,)
    s = 1.0 - a
    denom = np.maximum(s, 1e-8)
    binf = bias.astype(np.float64) / denom                   # (M,)

    i_idx = np.arange(P)
    # L_cat[j, m*128+i] = s_m * a_m^(i-j)  for j <= i, else 0
    diff = i_idx[None, :] - i_idx[:, None]                   # (j, i) = i - j
    lcat = np.zeros((P, M * P), dtype=np.float64)
    for m in range(M):
        Lm = np.where(diff >= 0, s[m] * np.power(a[m], np.maximum(diff, 0)), 0.0)
        lcat[:, m * P:(m + 1) * P] = Lm
    # dcat[p, m*128+i] = a_m^(i+1)   (broadcast over partitions)
    dcat = np.zeros((P, M * P), dtype=np.float64)
    for m in range(M):
        dcat[:, m * P:(m + 1) * P] = np.power(a[m], i_idx + 1)[None, :]
    # cinit[p, m] = -binf_m
    cinit = np.broadcast_to(-binf[None, :], (P, M)).copy()
    # cvec[n] = sum_m binf_m * sum_e W[e*M+m, n]  + b[n]
    Wf = W.astype(np.float64)
    cvec = b.astype(np.float64).copy()
    for m in range(M):
        cvec += binf[m] * Wf[m::M, :].sum(axis=0)
    cvecb = np.broadcast_to(cvec[None, :], (P, NO)).copy()
    # wt[g=et*M+m, e, :] = W[(et*128+e)*M + m, :]
    wt = W.reshape(ET, P, M, NO).transpose(0, 2, 1, 3).reshape(ET * M, P, NO)

    return (
        lcat.astype(np.float32),
        dcat.astype(np.float32),
        cinit.astype(np.float32),
        cvecb.astype(np.float32),
        np.ascontiguousarray(wt).astype(np.float32),
    )


def _build(mm_dtype="f32r", reps=1):
    import concourse.bacc as bacc
    import concourse.tile as tile
    import concourse.mybir as mybir

    key = ("nc", mm_dtype, reps)
    if key in _cache:
        return _cache[key]

    dt_mm = mybir.dt.float32r if mm_dtype == "f32r" else mybir.dt.float32
    f32 = mybir.dt.float32
    mult = mybir.AluOpType.mult
    add = mybir.AluOpType.add

    nc = bacc.Bacc("TRN2", target_bir_lowering=False, debug=False,
                   enable_asserts=False, num_devices=NC)

    xb = nc.dram_tensor("xb", [T, E], dt_mm, kind="ExternalInput").ap()
    wtd = nc.dram_tensor("wtd", [ET * M, P, NO], dt_mm, kind="ExternalInput").ap()
    lcat_d = nc.dram_tensor("lcat", [P, M * P], dt_mm, kind="ExternalInput").ap()
    dcat_d = nc.dram_tensor("dcat", [P, M * P], f32, kind="ExternalInput").ap()
    cinit_d = nc.dram_tensor("cinit", [P, M], f32, kind="ExternalInput").ap()
    cvecb_d = nc.dram_tensor("cvecb", [P, NO], f32, kind="ExternalInput").ap()
    out_d = nc.dram_tensor("out", [T, NO], f32, kind="ExternalOutput").ap()

    with tile.TileContext(nc) as tc:
        with (
            tc.tile_pool(name="consts", bufs=1) as cpool,
            tc.tile_pool(name="wpool", bufs=1) as wpool,
            tc.tile_pool(name="xpool", bufs=2) as xpool,
            tc.tile_pool(name="ypool", bufs=2) as ypool,
            tc.tile_pool(name="opool", bufs=2) as opool,
            tc.tile_pool(name="epsum", bufs=3, space="PSUM") as epsum,
            tc.tile_pool(name="ppsum", bufs=2, space="PSUM") as ppsum,
        ):
            lcat_t = cpool.tile([P, M * P], dt_mm)
            nc.sync.dma_start(out=lcat_t[:], in_=lcat_d[:])
            dcat_t = cpool.tile([P, M * P], f32)
            nc.sync.dma_start(out=dcat_t[:], in_=dcat_d[:])
            cinit_t = cpool.tile([P, M], f32)
            nc.sync.dma_start(out=cinit_t[:], in_=cinit_d[:])
            cvecb_t = cpool.tile([P, NO], f32)
            nc.sync.dma_start(out=cvecb_t[:], in_=cvecb_d[:])

            wt_t = []
            for g in range(ET * M):
                w = wpool.tile([P, NO], dt_mm, tag=f"w{g}")
                nc.sync.dma_start(out=w[:], in_=wtd[g])
                wt_t.append(w)

            def emit_rep():
                y_prev = None
                for c in range(CH):
                    xt = xpool.tile([P, E], dt_mm)
                    nc.sync.dma_start(out=xt[:], in_=xb[c * P:(c + 1) * P, :])

                    y_cur = []
                    for et in range(ET):
                        pe = epsum.tile([P, M * P], f32)
                        nc.tensor.matmul(
                            pe[:], xt[:, et * P:(et + 1) * P], lcat_t[:],
                            start=True, stop=True,
                        )
                        yt = ypool.tile([P, M * P], dt_mm, tag=f"y{et}")
                        for m in range(M):
                            sl = slice(m * P, (m + 1) * P)
                            if c == 0:
                                carry = cinit_t[:, m:m + 1]
                            else:
                                carry = y_prev[et][:, m * P + P - 1:m * P + P]
                            nc.vector.scalar_tensor_tensor(
                                out=yt[:, sl], in0=dcat_t[:, sl], scalar=carry,
                                in1=pe[:, sl], op0=mult, op1=add,
                            )
                        y_cur.append(yt)

                    ot = opool.tile([P, NO], f32)
                    for h in range(2):
                        po = ppsum.tile([P, NO // 2], f32, tag=f"po{h}")
                        k = 0
                        for et in range(ET):
                            for m in range(M):
                                nc.tensor.matmul(
                                    po[:],
                                    y_cur[et][:, m * P:(m + 1) * P],
                                    wt_t[et * M + m][:, h * (NO // 2):(h + 1) * (NO // 2)],
                                    start=(k == 0), stop=(k == ET * M - 1),
                                )
                                k += 1
                        nc.vector.tensor_add(
                            ot[:, h * (NO // 2):(h + 1) * (NO // 2)], po[:],
                            cvecb_t[:, h * (NO // 2):(h + 1) * (NO // 2)],
                        )
                    nc.sync.dma_start(out=out_d[c * P:(c + 1) * P, :], in_=ot[:])
                    y_prev = y_cur

            for _ in range(reps):
                emit_rep()

    nc.compile()
    _cache[key] = nc
    return nc


def _in_maps(inputs):
    x = np.ascontiguousarray(inputs["x"], dtype=np.float32)
    assert x.shape == (B, T, E), x.shape
    lcat, dcat, cinit, cvecb, wt = _host_consts(
        np.asarray(inputs["alpha_logits"], np.float32),
        np.asarray(inputs["bias"], np.float32),
        np.ascontiguousarray(inputs["W"], np.float32),
        np.asarray(inputs["b"], np.float32))
    in_maps = []
    for c in range(NC):
        in_maps.append({
            "xb": x[c],
            "wtd": wt,
            "lcat": lcat,
            "dcat": dcat,
            "cinit": cinit,
            "cvecb": cvecb,
        })
    return in_maps


def kernel(x, alpha_logits, bias, W, b):
    from concourse.bass_utils import run_bass_kernel_spmd

    in_maps = _in_maps({"x": x, "alpha_logits": alpha_logits, "bias": bias,
                        "W": W, "b": b})
    nc = _build()
    r = run_bass_kernel_spmd(nc, in_maps, list(range(NC)))
    out = np.stack([r.results[c]["out"] for c in range(NC)], axis=0)
    return out


# revision 12
# speedup vs baseline: 12.4101x; 1.2021x over previous
"""Trainium2 Bass kernel for the EMAMixer problem.

Semantics (stable form of the reference):
    for each m:  u_t = a_m * u_{t-1} + s_m * x_t,   u_{-1} = -bias_m / s_m
    y[t, e, m] = u_t[e] + bias_m / s_m
    out = y.reshape(B, T, E*M) @ W + b

Implementation per core (data-parallel over B, core c handles batch c):
  - T split into 16 chunks of 128. Per chunk and e-tile (128 features):
    one f32r matmul  x_chunk[:, et].T @ L_cat  ->  psum[e, (m,i)]
    where L_cat[j, m*128+i] = s_m * a_m^(i-j) for j<=i (in-chunk scan as matmul;
    output lands feature-major, i.e. already transposed for the projection).
  - carry across chunks: y = decay*(carry) + psum  via one DVE
    scalar_tensor_tensor per (e-tile, m); carry = col 127 of prev chunk's y.
  - projection: out_psum[i, n] += y_tile.T @ W_tile accumulated over all 32
    (e-tile, m) pairs, two 512-wide PSUM halves; then + cvec (bias + carry-shift
    correction) on DVE, DMA out.
All matmuls run in float32r (full-rate fp32 path on trn2).
"""

import numpy as np

B, T, E, M = 8, 2048, 1024, 4
P = 128                      # partition / chunk size
CH = T // P                  # 16 chunks
ET = E // P                  # 8 e-tiles
NC = 8                       # cores
NO = E                       # output features

_cache: dict = {}


def _sigmoid(x):
    return 1.0 / (1.0 + np.exp(-x))


def _host_consts(alpha_logits, bias, W, b):
    """Precompute small constant tensors in fp64, cast to fp32."""
    a = _sigmoid(alpha_logits.astype(np.float64))            # (M,)
    s = 1.0 - a
    denom = np.maximum(s, 1e-8)
    binf = bias.astype(np.float64) / denom                   # (M,)

    i_idx = np.arange(P)
    # L_cat[j, m*128+i] = s_m * a_m^(i-j)  for j <= i, else 0
    diff = i_idx[None, :] - i_idx[:, None]                   # (j, i) = i - j
    lcat = np.zeros((P, M * P), dtype=np.float64)
    for m in range(M):
        Lm = np.where(diff >= 0, s[m] * np.power(a[m], np.maximum(diff, 0)), 0.0)
        lcat[:, m * P:(m + 1) * P] = Lm
    # dcat[p, m*128+i] = a_m^(i+1)   (broadcast over partitions)
    dcat = np.zeros((P, M * P), dtype=np.float64)
    for m in range(M):
        dcat[:, m * P:(m + 1) * P] = np.power(a[m], i_idx + 1)[None, :]
    # cinit[p, m] = -binf_m
    cinit = np.broadcast_to(-binf[None, :], (P, M)).copy()
    # cvec[n] = sum_m binf_m * sum_e W[e*M+m, n]  + b[n]
    Wf = W.astype(np.float64)
    cvec = b.astype(np.float64).copy()
    for m in range(M):
        cvec += binf[m] * Wf[m::M, :].sum(axis=0)
    cvecb = np.broadcast_to(cvec[None, :], (P, NO)).copy()
    # wt[g=et*M+m, e, :] = W[(et*128+e)*M + m, :]
    wt = W.reshape(ET, P, M, NO).transpose(0, 2, 1, 3).reshape(ET * M, P, NO)

    return (
        lcat.astype(np.float32),
        dcat.astype(np.float32),
        cinit.astype(np.float32),
        cvecb.astype(np.float32),
        np.ascontiguousarray(wt).astype(np.float32),
    )


def _build(mm_dtype="f32r", reps=1, startup=2, ybufs=3, obufs=2, ebufs=2, pbufs=3):
    import concourse.bacc as bacc
    import concourse.tile as tile
    import concourse.mybir as mybir

    key = ("nc", mm_dtype, reps, startup, ybufs, obufs, ebufs, pbufs)
    if key in _cache:
        return _cache[key]

    dt_mm = mybir.dt.float32r if mm_dtype == "f32r" else mybir.dt.float32
    f32 = mybir.dt.float32
    mult = mybir.AluOpType.mult
    add = mybir.AluOpType.add

    nc = bacc.Bacc("TRN2", target_bir_lowering=False, debug=False,
                   enable_asserts=False, num_devices=NC)

    xb = nc.dram_tensor("xb", [T, E], dt_mm, kind="ExternalInput").ap()
    wtd = nc.dram_tensor("wtd", [ET * M, P, NO], dt_mm, kind="ExternalInput").ap()
    lcat_d = nc.dram_tensor("lcat", [P, M * P], dt_mm, kind="ExternalInput").ap()
    dcat_d = nc.dram_tensor("dcat", [P, M * P], f32, kind="ExternalInput").ap()
    cinit_d = nc.dram_tensor("cinit", [P, M], f32, kind="ExternalInput").ap()
    cvecb_d = nc.dram_tensor("cvecb", [P, NO], f32, kind="ExternalInput").ap()
    out_d = nc.dram_tensor("out", [T, NO], f32, kind="ExternalOutput").ap()

    with tile.TileContext(nc) as tc:
        with (
            tc.tile_pool(name="consts", bufs=1) as cpool,
            tc.tile_pool(name="wpool", bufs=1) as wpool,
            tc.tile_pool(name="xpool", bufs=2) as xpool,
            tc.tile_pool(name="ypool", bufs=ybufs) as ypool,
            tc.tile_pool(name="opool", bufs=obufs) as opool,
            tc.tile_pool(name="epsum", bufs=ebufs, space="PSUM") as epsum,
            tc.tile_pool(name="ppsum", bufs=pbufs, space="PSUM") as ppsum,
        ):
            lcat_t = cpool.tile([P, M * P], dt_mm)
            nc.sync.dma_start(out=lcat_t[:], in_=lcat_d[:])
            dcat_t = cpool.tile([P, M * P], f32)
            nc.sync.dma_start(out=dcat_t[:], in_=dcat_d[:])
            cinit_t = cpool.tile([P, M], f32)
            nc.sync.dma_start(out=cinit_t[:], in_=cinit_d[:])
            cvecb_t = cpool.tile([P, NO], f32)
            nc.sync.dma_start(out=cvecb_t[:], in_=cvecb_d[:])

            # W tiles, loaded half-by-half (h=0 halves of all 32 tiles first)
            # so chunk-0 projection can start after 8 MiB instead of 16.
            wt_t = []
            for g in range(ET * M):
                wt_t.append(wpool.tile([P, NO], dt_mm, tag=f"w{g}", name=f"w{g}"))
            for h in range(2):
                for g in range(ET * M):
                    hs = slice(h * (NO // 2), (h + 1) * (NO // 2))
                    nc.gpsimd.dma_start(out=wt_t[g][:, hs], in_=wtd[g][:, hs])

            HNO = NO // 2

            def emit_ema(c, y_prev):
                """EMA for chunk c: returns the 8 y tiles [e, (m,i)]."""
                xt = xpool.tile([P, E], dt_mm)
                nc.sync.dma_start(out=xt[:], in_=xb[c * P:(c + 1) * P, :])
                y_cur = []
                for et in range(ET):
                    pe = epsum.tile([P, M * P], f32)
                    nc.tensor.matmul(
                        pe[:], xt[:, et * P:(et + 1) * P], lcat_t[:],
                        start=True, stop=True,
                    )
                    yt = ypool.tile([P, M * P], dt_mm, tag=f"y{et}")
                    for m in range(M):
                        sl = slice(m * P, (m + 1) * P)
                        if c == 0:
                            carry = cinit_t[:, m:m + 1]
                        else:
                            carry = y_prev[et][:, m * P + P - 1:m * P + P]
                        nc.vector.scalar_tensor_tensor(
                            out=yt[:, sl], in0=dcat_t[:, sl], scalar=carry,
                            in1=pe[:, sl], op0=mult, op1=add,
                        )
                    y_cur.append(yt)
                return y_cur

            def emit_out(c, po_pair):
                ot = opool.tile([P, NO], f32)
                for h in range(2):
                    nc.vector.tensor_add(
                        ot[:, h * HNO:(h + 1) * HNO], po_pair[h][:],
                        cvecb_t[:, h * HNO:(h + 1) * HNO],
                    )
                nc.scalar.dma_start(out=out_d[c * P:(c + 1) * P, :], in_=ot[:])

            def emit_rep():
                S = min(startup, CH)
                # Startup: EMA for the first S chunks, then their projection
                # matmuls interleaved W-tile-major so the PE keeps busy while
                # W streams in.
                ys = []
                y_prev = None
                for c in range(S):
                    y_prev = emit_ema(c, y_prev)
                    ys.append(y_prev)
                pos = [[ppsum.tile([P, HNO], f32, tag=f"po{h}", name=f"po_s{c_}_{h}")
                        for h in range(2)] for c_ in range(S)]
                for h in range(2):
                    for g in range(ET * M):
                        et, m = divmod(g, M)
                        for c in range(S):
                            nc.tensor.matmul(
                                pos[c][h][:],
                                ys[c][et][:, m * P:(m + 1) * P],
                                wt_t[g][:, h * HNO:(h + 1) * HNO],
                                start=(g == 0), stop=(g == ET * M - 1),
                            )
                for c in range(S):
                    emit_out(c, pos[c])

                # Steady state
                for c in range(S, CH):
                    y_cur = emit_ema(c, y_prev)
                    po_pair = []
                    for h in range(2):
                        po = ppsum.tile([P, HNO], f32, tag=f"po{h}")
                        for g in range(ET * M):
                            et, m = divmod(g, M)
                            nc.tensor.matmul(
                                po[:],
                                y_cur[et][:, m * P:(m + 1) * P],
                                wt_t[g][:, h * HNO:(h + 1) * HNO],
                                start=(g == 0), stop=(g == ET * M - 1),
                            )
                        po_pair.append(po)
                    emit_out(c, po_pair)
                    y_prev = y_cur

            for _ in range(reps):
                emit_rep()

    nc.compile()
    _cache[key] = nc
    return nc


def _in_maps(inputs):
    x = np.ascontiguousarray(inputs["x"], dtype=np.float32)
    assert x.shape == (B, T, E), x.shape
    lcat, dcat, cinit, cvecb, wt = _host_consts(
        np.asarray(inputs["alpha_logits"], np.float32),
        np.asarray(inputs["bias"], np.float32),
        np.ascontiguousarray(inputs["W"], np.float32),
        np.asarray(inputs["b"], np.float32))
    in_maps = []
    for c in range(NC):
        in_maps.append({
            "xb": x[c],
            "wtd": wt,
            "lcat": lcat,
            "dcat": dcat,
            "cinit": cinit,
            "cvecb": cvecb,
        })
    return in_maps


def kernel(x, alpha_logits, bias, W, b):
    from concourse.bass_utils import run_bass_kernel_spmd

    in_maps = _in_maps({"x": x, "alpha_logits": alpha_logits, "bias": bias,
                        "W": W, "b": b})
    nc = _build()
    r = run_bass_kernel_spmd(nc, in_maps, list(range(NC)))
    out = np.stack([r.results[c]["out"] for c in range(NC)], axis=0)
    return out


# revision 21
# speedup vs baseline: 14.9244x; 1.2026x over previous
"""Trainium2 Bass kernel for the EMAMixer problem.

Semantics (stable form of the reference):
    for each m:  u_t = a_m * u_{t-1} + s_m * x_t,   u_{-1} = -bias_m / s_m
    y[t, e, m] = u_t[e] + bias_m / s_m
    out = y.reshape(B, T, E*M) @ W + b

Implementation per core (data-parallel over B, core c handles batch c):
  - T split into 16 chunks of 128. Per chunk and e-tile (128 features):
    one f32r matmul  x_chunk[:, et].T @ L_cat  ->  psum[e, (m,i)]
    where L_cat[j, m*128+i] = s_m * a_m^(i-j) for j<=i (in-chunk scan as matmul;
    output lands feature-major, i.e. already transposed for the projection).
  - carry across chunks: y = decay*(carry) + psum  via one DVE
    scalar_tensor_tensor per (e-tile, m); carry = col 127 of prev chunk's y.
  - projection: out_psum[i, n] += y_tile.T @ W_tile accumulated over all 32
    (e-tile, m) pairs, two 512-wide PSUM halves; then + cvec (bias + carry-shift
    correction) on DVE, DMA out.
All matmuls run in float32r (full-rate fp32 path on trn2).
"""

import numpy as np

B, T, E, M = 8, 2048, 1024, 4
P = 128                      # partition / chunk size
CH = T // P                  # 16 chunks
ET = E // P                  # 8 e-tiles
NC = 8                       # cores
NO = E                       # output features

_cache: dict = {}


def _sigmoid(x):
    return 1.0 / (1.0 + np.exp(-x))


def _host_consts(alpha_logits, bias, W, b):
    """Precompute small constant tensors in fp64, cast to fp32."""
    a = _sigmoid(alpha_logits.astype(np.float64))            # (M,)
    s = 1.0 - a
    denom = np.maximum(s, 1e-8)
    binf = bias.astype(np.float64) / denom                   # (M,)

    i_idx = np.arange(P)
    # L_cat[j, m*128+i] = s_m * a_m^(i-j)  for j <= i, else 0
    diff = i_idx[None, :] - i_idx[:, None]                   # (j, i) = i - j
    lcat = np.zeros((P, M * P), dtype=np.float64)
    for m in range(M):
        Lm = np.where(diff >= 0, s[m] * np.power(a[m], np.maximum(diff, 0)), 0.0)
        lcat[:, m * P:(m + 1) * P] = Lm
    # dcat[p, m*128+i] = a_m^(i+1)   (broadcast over partitions)
    dcat = np.zeros((P, M * P), dtype=np.float64)
    for m in range(M):
        dcat[:, m * P:(m + 1) * P] = np.power(a[m], i_idx + 1)[None, :]
    # cinit[p, m] = -binf_m
    cinit = np.broadcast_to(-binf[None, :], (P, M)).copy()
    # cvec[n] = sum_m binf_m * sum_e W[e*M+m, n]  + b[n]
    Wf = W.astype(np.float64)
    cvec = b.astype(np.float64).copy()
    for m in range(M):
        cvec += binf[m] * Wf[m::M, :].sum(axis=0)
    cvecb = np.broadcast_to(cvec[None, :], (P, NO)).copy()
    # wt[g=et*M+m, e, :] = W[(et*128+e)*M + m, :]
    wt = W.reshape(ET, P, M, NO).transpose(0, 2, 1, 3).reshape(ET * M, P, NO)

    return (
        lcat.astype(np.float32),
        dcat.astype(np.float32),
        cinit.astype(np.float32),
        cvecb.astype(np.float32),
        np.ascontiguousarray(wt).astype(np.float32),
    )


def _build(mm_dtype="f32r", reps=1, startup=2, ybufs=3, obufs=2, ebufs=2, pbufs=3):
    import concourse.bacc as bacc
    import concourse.tile as tile
    import concourse.mybir as mybir

    key = ("nc", mm_dtype, reps, startup, ybufs, obufs, ebufs, pbufs)
    if key in _cache:
        return _cache[key]

    dt_mm = mybir.dt.float32r if mm_dtype == "f32r" else mybir.dt.float32
    f32 = mybir.dt.float32
    mult = mybir.AluOpType.mult
    add = mybir.AluOpType.add

    nc = bacc.Bacc("TRN2", target_bir_lowering=False, debug=False,
                   enable_asserts=False, num_devices=NC)

    xb = nc.dram_tensor("xb", [T, E], dt_mm, kind="ExternalInput").ap()
    wtd = nc.dram_tensor("wtd", [ET * M, P, NO], dt_mm, kind="ExternalInput").ap()
    lcat_d = nc.dram_tensor("lcat", [P, M * P], dt_mm, kind="ExternalInput").ap()
    dcat_d = nc.dram_tensor("dcat", [P, M * P], f32, kind="ExternalInput").ap()
    cinit_d = nc.dram_tensor("cinit", [P, M], f32, kind="ExternalInput").ap()
    cvecb_d = nc.dram_tensor("cvecb", [P, NO], f32, kind="ExternalInput").ap()
    out_d = nc.dram_tensor("out", [T, NO], f32, kind="ExternalOutput").ap()

    with tile.TileContext(nc) as tc:
        with (
            tc.tile_pool(name="consts", bufs=1) as cpool,
            tc.tile_pool(name="wpool", bufs=1) as wpool,
            tc.tile_pool(name="xpool", bufs=2) as xpool,
            tc.tile_pool(name="ypool", bufs=ybufs) as ypool,
            tc.tile_pool(name="opool", bufs=obufs) as opool,
            tc.tile_pool(name="epsum", bufs=ebufs, space="PSUM") as epsum,
            tc.tile_pool(name="ppsum", bufs=pbufs, space="PSUM") as ppsum,
        ):
            def emit_xdma(c):
                xt = xpool.tile([P, E], dt_mm, name=f"x{c}", tag="x")
                nc.sync.dma_start(out=xt[:], in_=xb[c * P:(c + 1) * P, :])
                return xt

            # Sync-queue order: lcat + first x chunks first (they gate the
            # first EMA matmul), cvecb last (only needed ~40us in).
            lcat_t = cpool.tile([P, M * P], dt_mm)
            nc.sync.dma_start(out=lcat_t[:], in_=lcat_d[:])
            x0 = emit_xdma(0)
            x1 = emit_xdma(1)
            dcat_t = cpool.tile([P, M * P], f32)
            nc.sync.dma_start(out=dcat_t[:], in_=dcat_d[:])
            cinit_t = cpool.tile([P, M], f32)
            nc.sync.dma_start(out=cinit_t[:], in_=cinit_d[:])
            cvecb_t = cpool.tile([P, NO], f32)
            nc.sync.dma_start(out=cvecb_t[:], in_=cvecb_d[:])

            # W tiles, loaded half-by-half (h=0 halves of all 32 tiles first)
            # so chunk-0 projection can start after 8 MiB instead of 16.
            wt_t = []
            for g in range(ET * M):
                wt_t.append(wpool.tile([P, NO], dt_mm, tag=f"w{g}", name=f"w{g}"))
            for h in range(2):
                for g in range(ET * M):
                    hs = slice(h * (NO // 2), (h + 1) * (NO // 2))
                    nc.gpsimd.dma_start(out=wt_t[g][:, hs], in_=wtd[g][:, hs])

            HNO = NO // 2

            def emit_ema(c, y_prev, xt=None):
                """EMA for chunk c: returns the 8 y tiles [e, (m,i)]."""
                if xt is None:
                    xt = emit_xdma(c)
                y_cur = []
                for et in range(ET):
                    pe = epsum.tile([P, M * P], f32)
                    nc.tensor.matmul(
                        pe[:], xt[:, et * P:(et + 1) * P], lcat_t[:],
                        start=True, stop=True,
                    )
                    yt = ypool.tile([P, M * P], dt_mm, tag=f"y{et}")
                    for m in range(M):
                        sl = slice(m * P, (m + 1) * P)
                        if c == 0:
                            carry = cinit_t[:, m:m + 1]
                        else:
                            carry = y_prev[et][:, m * P + P - 1:m * P + P]
                        nc.vector.scalar_tensor_tensor(
                            out=yt[:, sl], in0=dcat_t[:, sl], scalar=carry,
                            in1=pe[:, sl], op0=mult, op1=add,
                        )
                    y_cur.append(yt)
                return y_cur

            def emit_out(c, po_pair):
                ot = opool.tile([P, NO], f32, name=f"os{c}", tag="o")
                for h in range(2):
                    nc.vector.tensor_add(
                        ot[:, h * HNO:(h + 1) * HNO], po_pair[h][:],
                        cvecb_t[:, h * HNO:(h + 1) * HNO],
                    )
                nc.scalar.dma_start(out=out_d[c * P:(c + 1) * P, :], in_=ot[:])

            def emit_proj(c, y):
                # per-half epilogue so the h0 add/DMA overlaps h1's matmuls
                ot = opool.tile([P, NO], f32, name=f"o{c}", tag="o")
                for h in range(2):
                    po = ppsum.tile([P, HNO], f32, tag=f"po{h}")
                    for g in range(ET * M):
                        et, m = divmod(g, M)
                        nc.tensor.matmul(
                            po[:],
                            y[et][:, m * P:(m + 1) * P],
                            wt_t[g][:, h * HNO:(h + 1) * HNO],
                            start=(g == 0), stop=(g == ET * M - 1),
                        )
                    hs = slice(h * HNO, (h + 1) * HNO)
                    nc.vector.tensor_add(ot[:, hs], po[:], cvecb_t[:, hs])
                    nc.scalar.dma_start(
                        out=out_d[c * P:(c + 1) * P, hs], in_=ot[:, hs])

            def emit_rep(x0=None, x1=None):
                # Software pipeline. Startup: EMA for chunks 0/1 (carry ops
                # interleaved e-tile-major across the two chunks), then their
                # projection interleaved W-tile-major so the PE keeps busy
                # while W streams in; ema(2) emitted between the two halves.
                # Steady state emits ema(c) BEFORE proj(c-1) so the DVE carry
                # chain for chunk c overlaps the chunk c-1 projection.
                if x0 is None:
                    x0 = emit_xdma(0)
                if x1 is None:
                    x1 = emit_xdma(1)
                # chunks 0/1 EMA fused e-tile-major so y0[et]/y1[et] pairs
                # complete together (startup proj consumes them pair-wise)
                y0, y1 = [], []
                for et in range(ET):
                    pe0 = epsum.tile([P, M * P], f32, name=f"pe0_{et}", tag="pe")
                    nc.tensor.matmul(pe0[:], x0[:, et * P:(et + 1) * P],
                                     lcat_t[:], start=True, stop=True)
                    pe1 = epsum.tile([P, M * P], f32, name=f"pe1_{et}", tag="pe")
                    nc.tensor.matmul(pe1[:], x1[:, et * P:(et + 1) * P],
                                     lcat_t[:], start=True, stop=True)
                    yt0 = ypool.tile([P, M * P], dt_mm, name=f"y0_{et}", tag=f"y{et}")
                    yt1 = ypool.tile([P, M * P], dt_mm, name=f"y1_{et}", tag=f"y{et}")
                    for m in range(M):
                        sl = slice(m * P, (m + 1) * P)
                        nc.vector.scalar_tensor_tensor(
                            out=yt0[:, sl], in0=dcat_t[:, sl],
                            scalar=cinit_t[:, m:m + 1], in1=pe0[:, sl],
                            op0=mult, op1=add)
                        nc.vector.scalar_tensor_tensor(
                            out=yt1[:, sl], in0=dcat_t[:, sl],
                            scalar=yt0[:, m * P + P - 1:m * P + P], in1=pe1[:, sl],
                            op0=mult, op1=add)
                    y0.append(yt0)
                    y1.append(yt1)
                ys = [y0, y1]
                pos = [[ppsum.tile([P, HNO], f32, tag=f"po{h}", name=f"po_s{c_}_{h}")
                        for h in range(2)] for c_ in range(2)]
                y2 = None
                for h in range(2):
                    for g in range(ET * M):
                        et, m = divmod(g, M)
                        for c in range(2):
                            nc.tensor.matmul(
                                pos[c][h][:],
                                ys[c][et][:, m * P:(m + 1) * P],
                                wt_t[g][:, h * HNO:(h + 1) * HNO],
                                start=(g == 0), stop=(g == ET * M - 1),
                            )
                    if h == 0 and CH > 2:
                        y2 = emit_ema(2, y1)
                for c in range(2):
                    emit_out(c, pos[c])

                if CH > 2:
                    y_prev, y_cur = y1, y2
                    for c in range(3, CH):
                        y_next = emit_ema(c, y_cur)
                        emit_proj(c - 1, y_cur)
                        y_prev, y_cur = y_cur, y_next
                    emit_proj(CH - 1, y_cur)

            for r in range(reps):
                emit_rep(x0 if r == 0 else None, x1 if r == 0 else None)

    nc.compile()
    _cache[key] = nc
    return nc


def _in_maps(inputs):
    x = np.ascontiguousarray(inputs["x"], dtype=np.float32)
    assert x.shape == (B, T, E), x.shape
    lcat, dcat, cinit, cvecb, wt = _host_consts(
        np.asarray(inputs["alpha_logits"], np.float32),
        np.asarray(inputs["bias"], np.float32),
        np.ascontiguousarray(inputs["W"], np.float32),
        np.asarray(inputs["b"], np.float32))
    in_maps = []
    for c in range(NC):
        in_maps.append({
            "xb": x[c],
            "wtd": wt,
            "lcat": lcat,
            "dcat": dcat,
            "cinit": cinit,
            "cvecb": cvecb,
        })
    return in_maps


def kernel(x, alpha_logits, bias, W, b):
    from concourse.bass_utils import run_bass_kernel_spmd

    in_maps = _in_maps({"x": x, "alpha_logits": alpha_logits, "bias": bias,
                        "W": W, "b": b})
    nc = _build()
    r = run_bass_kernel_spmd(nc, in_maps, list(range(NC)))
    out = np.stack([r.results[c]["out"] for c in range(NC)], axis=0)
    return out
